# revision 47
# baseline (speedup 1.0000x reference)
"""Trainium2 Bass kernel for nn_EnhancedFinGAT (4-layer GATv2 + GraphNorm + skip).

Strategy (8 NeuronCores, SPMD):
  - Nodes (and their incoming edges) are sharded by destination across the 8
    cores; per-core nodes are permuted into degree-bucket-major "slots" so the
    per-edge xr[dst] add becomes a broadcast access pattern (no second gather).
  - Per layer: local matmuls produce xl (normal layout, AllGathered into a
    DRAM table) and xr (transposed, SBUF).
  - Edge phase: ONE transposed dma_gather per ~2k-edge sub (dma_gather is
    Q7-descgen-bound at ~11.5us/2k idxs on HW, so the second normal-layout
    gather is replaced by PE transposes of the raw gathered data, with
    psum->sbuf copies alternating between DVE and ScalarE).  leaky-relu via
    the |z| decomposition (abs = bitwise-and on DVE), exp on ScalarE,
    per-128-edge mask matmuls accumulate weighted sums + softmax denominators
    in PSUM.  The whole edge phase is software-pipelined (emission order
    gather(i) | front(i-3) | mask(i-5)) so PE's in-order stream never waits
    on the current sub's DMA/DVE chain; broadcast operands are stored
    pairwise-duplicated (rep2) so DVE hits its 2x/4x perf modes.
  - GraphNorm stats via one AllReduce of (sum, sum-of-squares); skip matmul in
    transposed layout.
  - All inputs are packed into 3 blobs (f32/bf16/i16): per-exec dispatch under
    axon/PJRT costs ~45us per input tensor.
All heavy per-edge data is bf16; accumulations are f32.
"""

import os
import sys
import numpy as np

sys.path.insert(0, "/opt/trn_rl_repo")

import concourse.bass as bass
import concourse.bacc as bacc
import concourse.mybir as mybir
import concourse.tile as tile
import concourse.bass_utils as bass_utils
from concourse.masks import make_identity

fp32 = mybir.dt.float32
bf16 = mybir.dt.bfloat16
i16 = mybir.dt.int16

N, HID, L, H, CH = 10000, 256, 4, 4, 64
NCORES = 8
NPC = N // NCORES
EPS = 1e-5
P = 128


# ---------------------------------------------------------------- host prep

def _bucket_of(deg):
    # granularity 2 is the minimum the rep2 add's even-k requirement allows;
    # halves per-node padding vs granularity 4
    if deg <= 128:
        return max(2, ((deg + 1) // 2) * 2)
    for k in (144, 160, 176, 192, 208, 224, 240, 256, 288, 320, 384, 448, 512):
        if deg <= k:
            return k
    raise ValueError(deg)


def _cumcount(x):
    n = len(x)
    if n == 0:
        return np.zeros(0, np.int64)
    change = np.empty(n, dtype=bool)
    change[0] = True
    change[1:] = x[1:] != x[:-1]
    run_starts = np.flatnonzero(change)
    return np.arange(n) - run_starts[np.cumsum(change) - 1]


def build_layout(edge_index):
    src_g = np.concatenate([np.asarray(edge_index[0], np.int64), np.arange(N)])
    dst_g = np.concatenate([np.asarray(edge_index[1], np.int64), np.arange(N)])

    # degree-balanced node->core assignment: deal nodes (sorted by degree)
    # round-robin so the per-core bucket histograms almost coincide.
    deg_g = np.bincount(dst_g, minlength=N)
    order = np.argsort(-deg_g, kind="stable")
    core_nodes = [np.sort(order[c::NCORES]) for c in range(NCORES)]
    loc_of = np.zeros(N, np.int64)
    core_of_node = np.zeros(N, np.int64)
    for c in range(NCORES):
        loc_of[core_nodes[c]] = np.arange(NPC)
        core_of_node[core_nodes[c]] = c
    core_of = core_of_node[dst_g]

    per_core = []
    all_buckets = {}
    for c in range(NCORES):
        m = core_of == c
        s, d = src_g[m], loc_of[dst_g[m]]
        deg = np.bincount(d, minlength=NPC)
        buckets = np.array([_bucket_of(x) for x in deg])
        cnt = {}
        for k in buckets:
            cnt[int(k)] = cnt.get(int(k), 0) + 1
        for k, v in cnt.items():
            all_buckets[k] = max(all_buckets.get(k, 0), v)
        per_core.append((s, d, buckets))

    ks = sorted(all_buckets)
    M = {k: all_buckets[k] for k in ks}
    NL = ((sum(M.values()) + P - 1) // P) * P
    NT = NCORES * NL
    PAD_ROW = 0

    slot_bucket = np.zeros(NL, np.int64)
    off = 0
    bucket_slot_base = {}
    for k in ks:
        bucket_slot_base[k] = off
        slot_bucket[off:off + M[k]] = k
        off += M[k]

    n_tiles = NL // P
    SUB_CAP = 2048
    slot_edge_off = np.zeros(NL, np.int64)
    tile_edge_base = np.zeros(n_tiles + 1, np.int64)
    tile_subs = []  # per tile: list of (e_start, e_end, [(k, d0, m), ...])
    e = 0
    for t in range(n_tiles):
        tile_edge_base[t] = e
        subs = []
        sub_start = e
        sub_runs = []
        run = None  # (k, d0, m)
        for d in range(t * P, (t + 1) * P):
            k = int(slot_bucket[d])
            if k == 0:
                continue
            pad_now = ((e + P - 1) // P) * P
            if pad_now + k - sub_start > SUB_CAP:
                # close current sub before this node
                if run is not None:
                    sub_runs.append(run)
                    run = None
                e = pad_now
                subs.append((int(sub_start), int(e), sub_runs))
                sub_runs = []
                sub_start = e
            slot_edge_off[d] = e
            if run is not None and run[0] == k:
                run = (k, run[1], run[2] + 1)
            else:
                if run is not None:
                    sub_runs.append(run)
                run = (k, d, 1)
            e += k
        if run is not None:
            sub_runs.append(run)
        if e > sub_start or sub_runs:
            e = ((e + P - 1) // P) * P
            subs.append((int(sub_start), int(e), sub_runs))
        tile_subs.append(subs)
    tile_edge_base[n_tiles] = e
    E_pad = int(e)

    cores = []
    for c in range(NCORES):
        s, d, buckets = per_core[c]
        slot_of_node = np.full(NPC, -1, np.int64)
        next_free = dict(bucket_slot_base)
        for n_loc in np.argsort(buckets, kind="stable"):
            k = int(buckets[n_loc])
            slot_of_node[n_loc] = next_free[k]
            next_free[k] += 1
        order = np.argsort(slot_of_node[d], kind="stable")
        cores.append(dict(slot_of_node=slot_of_node,
                          s_sorted=s[order],
                          d_sorted_slot=slot_of_node[d][order]))

    g2p = np.zeros(N, np.int64)
    for c in range(NCORES):
        g2p[core_nodes[c]] = c * NL + cores[c]["slot_of_node"]

    for c in range(NCORES):
        src_slot = np.full(E_pad, PAD_ROW, np.int64)
        dst_slot = np.full(E_pad, -1, np.int64)
        d_sl = cores[c]["d_sorted_slot"]
        pos = slot_edge_off[d_sl] + _cumcount(d_sl)
        src_slot[pos] = g2p[cores[c]["s_sorted"]]
        dst_slot[pos] = d_sl
        cores[c]["src_slot"] = src_slot
        cores[c]["dst_slot_of_edge"] = dst_slot

    return dict(NL=int(NL), NT=int(NT), PAD_ROW=int(PAD_ROW), E_pad=E_pad,
                n_tiles=n_tiles, tile_edge_base=tile_edge_base,
                tile_subs=tile_subs, slot_edge_off=slot_edge_off,
                core_nodes=core_nodes, g2p=g2p), cores


def wrap_idx16(idx):
    n = len(idx)
    cols = (n + 15) // 16
    pad = np.zeros(cols * 16, np.int64)
    pad[:n] = idx
    w = pad.reshape(cols, 16).T.astype(np.int16)
    return np.tile(w, (8, 1))


def build_masks(layout, core):
    E_pad = layout["E_pad"]
    n_chunks = E_pad // P
    dst = core["dst_slot_of_edge"]
    masks = np.zeros((n_chunks, P, P), np.float32)
    for chn in range(n_chunks):
        d = dst[chn * P:(chn + 1) * P]
        rows = np.flatnonzero(d >= 0)
        masks[chn, rows, (d[d >= 0] % P)] = 1.0
    return masks


def att4_lhst(att_l, scale):
    out = np.zeros((2, P, P), np.float32)
    for b in range(2):
        for p in range(P):
            ch = 128 * b + p
            h = ch // CH
            out[b, p, h % 4::4] = scale * att_l[h, ch % CH]
    return out


def _to_bf16(x):
    import jax.numpy as jnp
    return np.asarray(jnp.asarray(x, jnp.bfloat16)).view(np.uint16)


# numpy bf16 arrays are passed as uint16 views?  Simpler: use ml_dtypes.
def to_bf16(x):
    import ml_dtypes
    return np.asarray(x, np.float32).astype(ml_dtypes.bfloat16)


def blob_parts_f32():
    return [
        ("br_t", (L, 2, 128, 1)),
        ("bsk", (2, 128, 1)),
        ("cbA", (L, 2, 128, 1)),
        ("cbB", (L, 2, 128, 1)),
        ("gnw_t", (L, 2, 128, 1)),
        ("gnb_t", (L, 2, 128, 1)),
        ("fcb", (1, 1)),
    ]


def blob_parts_bf16(NCH, NL):
    return [
        ("x0t", (2, 128, NL)),
        ("wl", (L, 2, 128, 256)),
        ("wr", (L, 2, 128, 256)),
        ("wsk", (2, 2, 128, 128)),
        ("bl", (L, 1, 256)),
        ("fcw_t", (2, 128, 1)),
        ("a4z", (L, 2, 128, 128)),
        ("a4a", (L, 2, 128, 128)),
        ("masks", (128, NCH, 128)),
    ]


def pack_blob(parts, arrays, dtype):
    out = []
    for name, shape in parts:
        a = np.asarray(arrays[name])
        assert tuple(a.shape) == tuple(shape), (name, a.shape, shape)
        out.append(np.ascontiguousarray(a).reshape(-1))
    return np.concatenate(out).astype(dtype, copy=False)


def prep_inputs(inputs):
    """Returns (layout, in_maps) — one dict per core."""
    layout, cores = build_layout(np.asarray(inputs["edge_index"]))
    NL, E_pad = layout["NL"], layout["E_pad"]

    x = np.asarray(inputs["x"], np.float32)
    lw = np.asarray(inputs["lin_l_w"], np.float32)
    lb = np.asarray(inputs["lin_l_b"], np.float32)
    rw = np.asarray(inputs["lin_r_w"], np.float32)
    rb = np.asarray(inputs["lin_r_b"], np.float32)
    att = np.asarray(inputs["att"], np.float32)
    cb = np.asarray(inputs["conv_bias"], np.float32)
    gnw = np.asarray(inputs["gn_weight"], np.float32)
    gnb = np.asarray(inputs["gn_bias"], np.float32)
    gnm = np.asarray(inputs["gn_mean_scale"], np.float32)
    skw = np.asarray(inputs["skip_w"], np.float32)
    skb = np.asarray(inputs["skip_b"], np.float32)
    fcw = np.asarray(inputs["fc_w"], np.float32)
    fcb = np.asarray(inputs["fc_b"], np.float32)

    # layer-stacked common weights (same on all cores)
    wl_in = lw.reshape(L, 2, 128, 256)                    # [l, bi, 128, 256]
    wr_in = rw.reshape(L, 2, 128, 256)
    wsk_in = skw.reshape(2, 128, 2, 128).transpose(0, 2, 1, 3)  # [bi, bo, 128, 128]
    bl_in = lb.reshape(L, 1, 256)
    br_t = rb.reshape(L, 2, 128, 1)
    bsk_eff = (skb - skw.sum(axis=0)).reshape(2, 128, 1)
    a4z = np.stack([att4_lhst(att[l], 0.6) for l in range(L)])  # [L, 2, 128, 128]
    a4a = np.stack([att4_lhst(att[l], 0.4) for l in range(L)])
    cbA = (cb * (1.0 - gnm)).reshape(L, 2, 128, 1)
    cbB = np.broadcast_to((-gnm / float(N)).reshape(L, 2, 128, 1), (L, 2, 128, 1))
    gnw_t = gnw.reshape(L, 2, 128, 1)
    gnb_t = gnb.reshape(L, 2, 128, 1)
    fcw_t = fcw.reshape(2, 128, 1)
    fcb_in = fcb.reshape(1, 1)

    common = dict(
        wl=wl_in, wr=wr_in, wsk=wsk_in.copy(), bl=bl_in, br_t=br_t,
        bsk=bsk_eff, a4z=to_bf16(a4z), a4a=to_bf16(a4a),
        cbA=cbA, cbB=np.asarray(cbB, np.float32).copy(), gnw_t=gnw_t, gnb_t=gnb_t,
        fcw_t=fcw_t, fcb=fcb_in,
    )

    import ml_dtypes
    NCH = E_pad // P
    fparts = blob_parts_f32()
    bparts = blob_parts_bf16(NCH, NL)
    in_maps = []
    for c in range(NCORES):
        core = cores[c]
        x0 = np.zeros((NL, HID), np.float32)
        x0[core["slot_of_node"]] = x[layout["core_nodes"][c]]
        x0t = x0.T.reshape(2, 128, NL).copy()
        arrays = dict(common)
        arrays["x0t"] = x0t
        arrays["masks"] = to_bf16(
            build_masks(layout, core).transpose(1, 0, 2).copy())
        m = dict(
            blob_f=pack_blob(fparts, arrays, np.float32),
            blob_b=pack_blob(bparts, arrays, ml_dtypes.bfloat16),
            srcw=wrap_idx16(core["src_slot"]),
        )
        in_maps.append(m)

    layout["cores"] = cores
    return layout, in_maps


# ---------------------------------------------------------------- device build

def build_program(layout, n_layers=L, do_edges=True, do_coll=True,
                  sbuf_table=False, abs_on_dve=True, use_f32r=False,
                  skew_front=3, skew_mask=5, gn_gather=False,
                  bufs_gt=4, bufs_gn=3, bufs_mk=5, bufs_wg=4,
                  gather_only=False):
    NL, NT, E_pad = layout["NL"], layout["NT"], layout["E_pad"]
    n_tiles = layout["n_tiles"]
    teb = layout["tile_edge_base"]
    EW = (E_pad + 15) // 16
    NCH = E_pad // P
    SUBMAX = 128 * max(
        (s[1] - s[0]) // 128 for subs in layout["tile_subs"] for s in subs)

    def mmT(x):
        # bitcast f32 matmul operands to float32r (1 cycle/row at >=256 cols)
        return x.bitcast(mybir.dt.float32r) if use_f32r else x

    nc = bacc.Bacc("TRN2", target_bir_lowering=False, debug=False,
                   num_devices=NCORES)

    # ---- I/O: consolidated into 3 blobs — per-exec dispatch costs ~45us
    # per input tensor under axon/PJRT, so fewer inputs = faster.
    fparts = blob_parts_f32()
    bparts = blob_parts_bf16(NCH, NL)
    totf = sum(np.prod(s) for _, s in fparts)
    totb = sum(np.prod(s) for _, s in bparts)
    blob_f = nc.dram_tensor("blob_f", [int(totf)], fp32, kind="ExternalInput")
    blob_b = nc.dram_tensor("blob_b", [int(totb)], bf16, kind="ExternalInput")
    srcw = nc.dram_tensor("srcw", [128, EW], i16, kind="ExternalInput")

    def _views(blob, parts):
        out, off = {}, 0
        for name, shape in parts:
            n = int(np.prod(shape))
            ap = blob[off:off + n]
            if len(shape) > 1:
                names = ["d%d" % i for i in range(len(shape))]
                pat = "(" + " ".join(names) + ") -> " + " ".join(names)
                ap = ap.rearrange(pat, **{nm: int(s) for nm, s in
                                          zip(names, shape)})
            out[name] = ap
            off += n
        return out

    fv = _views(blob_f, fparts)
    bv = _views(blob_b, bparts)
    x0t, wl, wr, wsk = bv["x0t"], bv["wl"], bv["wr"], bv["wsk"]
    bl, fcw_t = bv["bl"], bv["fcw_t"]
    br_t, bsk = fv["br_t"], fv["bsk"]
    cbA, cbB, gnw_t, gnb_t = fv["cbA"], fv["cbB"], fv["gnw_t"], fv["gnb_t"]
    fcb = fv["fcb"]
    a4z, a4a, masks = bv["a4z"], bv["a4a"], bv["masks"]

    y_out = nc.dram_tensor("y", [1, NL], fp32, kind="ExternalOutput")

    # ---- internal DRAM
    xl_bounce = nc.dram_tensor("xl_bounce", [NL, 256], bf16, kind="Internal")
    st_in = nc.dram_tensor("st_in", [256, 2], fp32, kind="Internal")
    st_out = nc.dram_tensor("st_out", [256, 2], fp32, kind="Internal",
                            addr_space="Shared")
    xlt_sh = nc.dram_tensor("xlt_sh", [NT, 256], bf16, kind="Internal",
                            addr_space="Shared")

    groups = [list(range(NCORES))]

    with tile.TileContext(nc) as tc:
        with tc.tile_pool(name="persist", bufs=1) as pp, \
             tc.tile_pool(name="work", bufs=2) as wp, \
             tc.tile_pool(name="single", bufs=1) as sp, \
             tc.tile_pool(name="psum", bufs=2, space="PSUM") as psp:

            # ---------------- constants / persistent state
            ident128 = pp.tile([128, 128], fp32, tag="id128", name="id128")
            make_identity(nc, ident128[:])
            ident128b = pp.tile([128, 128], bf16, tag="id128b", name="id128b")
            nc.vector.tensor_copy(ident128b[:], ident128[:])
            ident16 = pp.tile([16, 16], bf16, tag="id16", name="id16")
            nc.gpsimd.memset(ident16[:], 0)
            # identity via iota trick: reuse make_identity on a f32 tile & copy
            id16f = pp.tile([16, 16], fp32, tag="id16f", name="id16f")
            make_identity(nc, id16f[:])
            nc.vector.tensor_copy(ident16[:], id16f[:])

            ones_row = pp.tile([1, 128], bf16, tag="ones", name="ones")
            eps_t = pp.tile([128, 1], fp32, tag="epsT", name="epsT")
            nc.gpsimd.memset(eps_t[:], EPS)
            nc.gpsimd.memset(ones_row[:], 1.0)

            xT = [pp.tile([128, NL], bf16, tag=f"xT{b}", name=f"xT{b}") for b in range(2)]
            xrT = [pp.tile([128, NL], bf16, tag=f"xrT{b}", name=f"xrT{b}") for b in range(2)]
            # xr duplicated pairwise so the per-edge broadcast add has a
            # contiguous last dim (enables DVE 2x/4x perf modes)
            xr2 = [pp.tile([128, NL, 2], bf16, tag=f"xr2{b}", name=f"xr2{b}")
                   for b in range(2)]
            outT = [pp.tile([128, NL], fp32, tag=f"outT{b}", name=f"outT{b}") for b in range(2)]
            hT = [pp.tile([128, NL], bf16, tag=f"hT{b}", name=f"hT{b}") for b in range(2)]
            xl_sb = pp.tile([128, (NL // 128) * 256], bf16, tag="xlsb", name="xlsb")
            srcw_sb = pp.tile([128, EW], i16, tag="srcsb", name="srcsb")
            nc.sync.dma_start(out=srcw_sb[:], in_=srcw[:])
            # SBUF-resident xl table: partition p holds tokens t with t%128==p,
            # rank-major (rank = t//128), 512B per rank stripe.
            xltab = (pp.tile([128, (NT // 128) * 256], bf16, tag="xltab",
                             name="xltab")
                     if sbuf_table else None)

            for b in range(2):
                nc.sync.dma_start(out=xT[b][:], in_=x0t[b])

            # per-layer weight staging
            wl_s = pp.tile([128, 2, 256], bf16, tag="wls", name="wls")
            wr_s = pp.tile([128, 2, 256], bf16, tag="wrs", name="wrs")
            wsk_s = pp.tile([128, 2, 2, 128], bf16, tag="wsks", name="wsks")
            bl_s = pp.tile([1, 256], bf16, tag="bls", name="bls")
            br_s = pp.tile([128, 2], fp32, tag="brs", name="brs")
            bsk_s = pp.tile([128, 2], fp32, tag="bsks", name="bsks")
            a4z_s = pp.tile([128, 2, 128], bf16, tag="a4zs", name="a4zs")
            a4a_s = pp.tile([128, 2, 128], bf16, tag="a4as", name="a4as")
            cbA_s = pp.tile([128, 2], fp32, tag="cbAs", name="cbAs")
            cbB_s = pp.tile([128, 2], fp32, tag="cbBs", name="cbBs")
            gnw_s = pp.tile([128, 2], fp32, tag="gnws", name="gnws")
            gnb_s = pp.tile([128, 2], fp32, tag="gnbs", name="gnbs")

            for b in range(2):
                nc.sync.dma_start(out=wsk_s[:, b], in_=wsk[b].rearrange("o p c -> p o c"))
            nc.sync.dma_start(out=bsk_s[:], in_=bsk.rearrange("b p o -> p (b o)"))

            def layer(l):
                # ---- stage layer weights
                nc.sync.dma_start(out=wl_s[:], in_=wl[l].rearrange("b p c -> p b c"))
                nc.sync.dma_start(out=wr_s[:], in_=wr[l].rearrange("b p c -> p b c"))
                nc.sync.dma_start(out=bl_s[:], in_=bl[l])
                nc.sync.dma_start(out=br_s[:], in_=br_t[l].rearrange("b p o -> p (b o)"))
                nc.sync.dma_start(out=a4z_s[:], in_=a4z[l].rearrange("b p c -> p b c"))
                nc.sync.dma_start(out=a4a_s[:], in_=a4a[l].rearrange("b p c -> p b c"))
                nc.sync.dma_start(out=cbA_s[:], in_=cbA[l].rearrange("b p o -> p (b o)"))
                nc.sync.dma_start(out=cbB_s[:], in_=cbB[l].rearrange("b p o -> p (b o)"))
                nc.sync.dma_start(out=gnw_s[:], in_=gnw_t[l].rearrange("b p o -> p (b o)"))
                nc.sync.dma_start(out=gnb_s[:], in_=gnb_t[l].rearrange("b p o -> p (b o)"))

                # ---- xl (normal layout) and xr (transposed) from x_T
                for t in range(NL // 128):
                    ps = psp.tile([128, 512], fp32, tag="lg", name="lg", bufs=2)
                    for bi in range(2):
                        nc.tensor.matmul(out=ps[:, :256],
                                         lhsT=mmT(xT[bi][:, t * 128:(t + 1) * 128]),
                                         rhs=mmT(wl_s[:, bi]),
                                         start=(bi == 0), stop=False)
                    nc.tensor.matmul(out=ps[:, :256], lhsT=mmT(ones_row[:]),
                                     rhs=mmT(bl_s[:]), start=False, stop=True)
                    nc.vector.tensor_copy(
                        xl_sb[:, t * 256:(t + 1) * 256], ps[:, :256])
                nc.sync.dma_start(
                    out=xl_bounce[:].rearrange("(t p) c -> p t c", p=128),
                    in_=xl_sb[:].rearrange("p (t c) -> p t c", c=256))

                # xr transposed: out block bo over node chunks of 512
                for bo in range(2):
                    for ch0 in range(0, NL, 512):
                        cw = min(512, NL - ch0)
                        ps = psp.tile([128, 512], fp32, tag="wsum", name="wsum")
                        for bi in range(2):
                            nc.tensor.matmul(
                                out=ps[:, :cw],
                                lhsT=mmT(wr_s[:, bi, bo * 128:(bo + 1) * 128]),
                                rhs=mmT(xT[bi][:, ch0:ch0 + cw]),
                                start=(bi == 0), stop=(bi == 1))
                        nc.scalar.activation(
                            out=xrT[bo][:, ch0:ch0 + cw], in_=ps[:, :cw],
                            func=mybir.ActivationFunctionType.Identity,
                            bias=br_s[:, bo:bo + 1], scale=1.0)
                    for q in range(2):
                        nc.vector.tensor_copy(xr2[bo][:, :, q], xrT[bo][:])

                # ---- AllGather xl into the table
                if do_coll:
                    nc.gpsimd.collective_compute(
                        "AllGather", mybir.AluOpType.bypass,
                        replica_groups=groups,
                        ins=[xl_bounce[:]],
                        outs=[xlt_sh[:]],
                    )
                if sbuf_table:
                    # one contiguous-partition DMA: token t -> partition t%128,
                    # rank t//128 (each core block is NL/128 consecutive ranks)
                    nc.sync.dma_start(
                        out=xltab[:].rearrange("p (r f) -> p r f", f=256),
                        in_=xlt_sh[:].rearrange("(r p) f -> p r f", p=128))

                # ---- edge phase: software-pipelined over all subs so PE's
                # in-order stream never stalls on the current sub's chain.
                # iteration i emits: gather(i) | front(i-1) | mask(i-2)
                if not do_edges:
                    for b in range(2):
                        nc.gpsimd.memset(outT[b][:], 0.0)
                subs_flat = []
                for t in range(n_tiles if do_edges else 0):
                    for s in layout["tile_subs"][t]:
                        subs_flat.append((t, s))
                nsf = len(subs_flat)
                tile_nch = [sum((s[1] - s[0]) // 128 for s in layout["tile_subs"][t])
                            for t in range(n_tiles)]
                st = {}
                pw_st = {}  # tile -> [pw tile, chunk_i]

                def stage_gather(i):
                    t, (e0, e1, runs) = subs_flat[i]
                    et = e1 - e0
                    nch = et // 128
                    gt = wp.tile([128, 2 * et], bf16, tag="gt", name="gt",
                                 padded_shape=[128, 2 * SUBMAX], bufs=bufs_gt)
                    if sbuf_table:
                        nc.gpsimd.dma_gather(
                            out_ap=gt[:].rearrange("p (b e) -> p b e", b=2),
                            in_ap=xltab[:],
                            idxs_ap=srcw_sb[:, e0 // 16:e1 // 16],
                            num_idxs=et, num_idxs_reg=et, elem_size=256,
                            transpose=True, single_packet=False,
                            sbuf_tokens_per_rank=128,
                            sbuf_free_dim_per_rank=512)
                    else:
                        nc.gpsimd.dma_gather(
                            out_ap=gt[:].rearrange("p (b e) -> p b e", b=2),
                            in_ap=xlt_sh[:],
                            idxs_ap=srcw_sb[:, e0 // 16:e1 // 16],
                            num_idxs=et, num_idxs_reg=et, elem_size=256,
                            transpose=True, single_packet=False)
                    gn = None
                    if gn_gather:
                        gn = wp.tile([128, nch, 256], bf16, tag="gn", name="gn",
                                     padded_shape=[128, SUBMAX // 128, 256],
                                     bufs=bufs_gn)
                        nc.gpsimd.dma_gather(
                            out_ap=gn[:], in_ap=xlt_sh[:],
                            idxs_ap=srcw_sb[:, e0 // 16:e1 // 16],
                            num_idxs=et, num_idxs_reg=et, elem_size=256,
                            transpose=False, single_packet=False)
                    mk = wp.tile([128, nch, 128], bf16, tag="mk", name="mk",
                                 padded_shape=[128, SUBMAX // 128, 128], bufs=bufs_mk)
                    nc.sync.dma_start(
                        out=mk[:], in_=masks[:, e0 // 128:e1 // 128, :])
                    st[i] = dict(t=t, e0=e0, et=et, nch=nch, runs=runs,
                                 gt=gt, gn=gn, mk=mk)

                def stage_front_a(i):
                    # derive the normal layout on-chip: PE-transpose the raw
                    # transposed-gather data, 2 chunks per psum bank;
                    # psum->sbuf copies alternate between DVE and ScalarE.
                    # Runs one iteration before stage_front so the consumers
                    # there never wait on this sub's PE work.
                    s = st[i]
                    et, nch, gt = s["et"], s["nch"], s["gt"]
                    if gn_gather:
                        return
                    gn = wp.tile([128, nch, 256], bf16, tag="gn", name="gn",
                                 padded_shape=[128, SUBMAX // 128, 256],
                                 bufs=bufs_gn)
                    for g in range(0, nch, 2):
                        gw = min(2, nch - g)
                        gp = psp.tile([128, 512], bf16, tag="gn2", name="gn2")
                        for j in range(gw):
                            for b in range(2):
                                nc.tensor.transpose(
                                    out=gp[:, j * 256 + b * 128:
                                           j * 256 + (b + 1) * 128],
                                    in_=gt[:, b * et + (g + j) * 128:
                                           b * et + (g + j + 1) * 128],
                                    identity=ident128b[:])
                        dst = gn[:, g:g + gw, :]
                        if (g // 2) % 2 == 0:
                            nc.vector.tensor_copy(dst, gp[:, :gw * 256]
                                                  .rearrange("p (n f) -> p n f", f=256))
                        else:
                            nc.scalar.activation(
                                out=dst, in_=gp[:, :gw * 256]
                                .rearrange("p (n f) -> p n f", f=256),
                                func=mybir.ActivationFunctionType.Identity)
                    s["gn"] = gn

                def stage_front(i):
                    s = st[i]
                    et, nch, gt, gn = s["et"], s["nch"], s["gt"], s["gn"]
                    za = wp.tile([128, 2 * et], bf16, tag="za", name="za",
                                 padded_shape=[128, 2 * SUBMAX])
                    w16 = wp.tile([16, et], bf16, tag="w16", name="w16",
                                  padded_shape=[16, SUBMAX])

                    # z = g + xr[dst] via bucket-broadcast (rep2 layout keeps
                    # the last dim contiguous -> DVE fast mode)
                    for b in range(2):
                        for (k, d0, m) in s["runs"]:
                            off = int(layout["slot_edge_off"][d0] - s["e0"])
                            seg = gt[:, b * et + off:b * et + off + m * k] \
                                .rearrange("p (m k2 two) -> p m k2 two",
                                           m=m, two=2)
                            in1 = xr2[b][:, d0:d0 + m, :] \
                                .rearrange("p m (two one) -> p m one two", one=1) \
                                .broadcast_to([128, m, k // 2, 2])
                            nc.vector.tensor_tensor(
                                out=seg, in0=seg, in1=in1,
                                op=mybir.AluOpType.add)
                    if abs_on_dve:
                        nc.vector.tensor_scalar(
                            out=za[:].bitcast(i16), in0=gt[:, :2 * et].bitcast(i16),
                            scalar1=0x7FFF, scalar2=None,
                            op0=mybir.AluOpType.bitwise_and)
                    else:
                        for b in range(2):
                            nc.scalar.activation(
                                out=za[:, b * et:(b + 1) * et],
                                in_=gt[:, b * et:(b + 1) * et],
                                func=mybir.ActivationFunctionType.Abs)

                    for ch0 in range(0, et, 512):
                        cw = min(512, et - ch0)
                        lg = psp.tile([128, 512], fp32, tag="lg", name="lg", bufs=2)
                        nc.tensor.matmul(out=lg[:, :cw], lhsT=a4z_s[:, 0],
                                         rhs=gt[:, ch0:ch0 + cw],
                                         start=True, stop=False)
                        nc.tensor.matmul(out=lg[:, :cw], lhsT=a4z_s[:, 1],
                                         rhs=gt[:, et + ch0:et + ch0 + cw],
                                         start=False, stop=False)
                        nc.tensor.matmul(out=lg[:, :cw], lhsT=a4a_s[:, 0],
                                         rhs=za[:, ch0:ch0 + cw],
                                         start=False, stop=False)
                        nc.tensor.matmul(out=lg[:, :cw], lhsT=a4a_s[:, 1],
                                         rhs=za[:, et + ch0:et + ch0 + cw],
                                         start=False, stop=True)
                        nc.scalar.activation(
                            out=w16[:, ch0:ch0 + cw], in_=lg[:16, :cw],
                            func=mybir.ActivationFunctionType.Exp)
                    wn = wp.tile([128, nch, 4, 2], bf16, tag="wn", name="wn",
                                 padded_shape=[128, SUBMAX // 128, 4, 2])
                    wg = wp.tile([128, nch, 260], bf16, tag="wg", name="wg",
                                 padded_shape=[128, SUBMAX // 128, 260], bufs=bufs_wg)
                    wt = psp.tile([128, (SUBMAX // 128) * 16], bf16, tag="wt", name="wt")
                    for n in range(nch):
                        nc.tensor.transpose(
                            out=wt[:, n * 16:(n + 1) * 16],
                            in_=w16[:, n * 128:(n + 1) * 128],
                            identity=ident16[:])
                    wtv = wt[:, :nch * 16].rearrange("p (n s) -> p n s", s=16)
                    for q in range(2):
                        nc.vector.tensor_copy(wn[:, :, :, q], wtv[:, :, 0:4])

                    nc.vector.tensor_tensor(
                        out=wg[:, :, 0:256].rearrange(
                            "p n (f c2 two) -> p n f c2 two", f=4, two=2),
                        in0=gn[:].rearrange(
                            "p n (f c2 two) -> p n f c2 two", f=4, two=2),
                        in1=wn[:].rearrange(
                            "p n f (two one) -> p n f one two", one=1)
                            .broadcast_to([128, nch, 4, 32, 2]),
                        op=mybir.AluOpType.mult)
                    nc.vector.tensor_copy(wg[:, :, 256:260], wn[:, :, :, 0])
                    s["wg"] = wg

                def stage_mask(i):
                    s = st.pop(i)
                    t = s["t"]
                    if t not in pw_st:
                        pw_st[t] = [psp.tile([128, 512], fp32, tag="wsum",
                                             name="wsum"), 0]
                    pw, ci = pw_st[t]
                    for n in range(s["nch"]):
                        nc.tensor.matmul(out=pw[:, :260], lhsT=s["mk"][:, n],
                                         rhs=s["wg"][:, n],
                                         start=(ci == 0),
                                         stop=(ci == tile_nch[t] - 1))
                        ci += 1
                    pw_st[t][1] = ci
                    if ci == tile_nch[t]:
                        close_tile(t, pw)
                        del pw_st[t]

                def close_tile(t, pw):
                    srec = wp.tile([128, 4], fp32, tag="srec", name="srec")
                    nc.vector.tensor_scalar(
                        out=srec[:], in0=pw[:, 256:260], scalar1=1e-20,
                        scalar2=None, op0=mybir.AluOpType.add)
                    nc.vector.reciprocal(srec[:], srec[:])
                    outn = wp.tile([128, 256], fp32, tag="outn", name="outn")
                    nc.vector.tensor_tensor(
                        out=outn[:].rearrange("p (f c) -> p f c", f=4),
                        in0=pw[:, 0:256].rearrange("p (f c) -> p f c", f=4),
                        in1=srec[:].to_broadcast([128, 4, 64]),
                        op=mybir.AluOpType.mult)
                    for b in range(2):
                        tp = psp.tile([128, 128], fp32, tag="wt", name="wt")
                        nc.tensor.transpose(
                            out=tp[:], in_=outn[:, b * 128:(b + 1) * 128],
                            identity=ident128[:])
                        nc.vector.tensor_copy(
                            outT[b][:, t * 128:(t + 1) * 128], tp[:])

                if gather_only:
                    for b in range(2):
                        nc.gpsimd.memset(outT[b][:], 0.0)
                    for i in range(nsf):
                        stage_gather(i)
                        st.pop(i)
                else:
                    skew_fa = skew_front - 1
                    for i in range(nsf + skew_mask):
                        if i < nsf:
                            stage_gather(i)
                        if 0 <= i - skew_fa < nsf:
                            stage_front_a(i - skew_fa)
                        if 0 <= i - skew_front < nsf:
                            stage_front(i - skew_front)
                        if 0 <= i - skew_mask < nsf:
                            stage_mask(i - skew_mask)

                # ---- GraphNorm stats (global) + h + skip
                s12 = sp.tile([128, 4], fp32, tag="s12", name="s12")  # [S1b0 S2b0 S1b1 S2b1]
                sq = sp.tile([128, NL], fp32, tag="sq", name="sq")
                for b in range(2):
                    nc.vector.tensor_reduce(
                        out=s12[:, 2 * b:2 * b + 1], in_=outT[b][:],
                        axis=mybir.AxisListType.X, op=mybir.AluOpType.add)
                    nc.vector.tensor_tensor(out=sq[:], in0=outT[b][:],
                                            in1=outT[b][:],
                                            op=mybir.AluOpType.mult)
                    nc.vector.tensor_reduce(
                        out=s12[:, 2 * b + 1:2 * b + 2], in_=sq[:],
                        axis=mybir.AxisListType.X, op=mybir.AluOpType.add)
                for b in range(2):
                    nc.sync.dma_start(out=st_in[b * 128:(b + 1) * 128, :],
                                      in_=s12[:, 2 * b:2 * b + 2])
                if do_coll:
                    nc.gpsimd.collective_compute(
                        "AllReduce", mybir.AluOpType.add,
                        replica_groups=groups, ins=[st_in[:]], outs=[st_out[:]])
                else:
                    nc.sync.dma_start(out=st_out[:], in_=st_in[:])
                s12g = sp.tile([128, 4], fp32, tag="s12g", name="s12g")
                for b in range(2):
                    nc.sync.dma_start(out=s12g[:, 2 * b:2 * b + 2],
                                      in_=st_out[b * 128:(b + 1) * 128, :])

                c1 = sp.tile([128, 2], fp32, tag="c1", name="c1")
                var = sp.tile([128, 2], fp32, tag="var", name="var")
                rstd = sp.tile([128, 2], fp32, tag="rstd", name="rstd")
                f_ = sp.tile([128, 2], fp32, tag="f_", name="f_")
                for b in range(2):
                    S1 = s12g[:, 2 * b:2 * b + 1]
                    S2 = s12g[:, 2 * b + 1:2 * b + 2]
                    # c1 = A + B*S1
                    nc.vector.tensor_tensor(out=c1[:, b:b + 1],
                                            in0=S1, in1=cbB_s[:, b:b + 1],
                                            op=mybir.AluOpType.mult)
                    nc.vector.tensor_tensor(out=c1[:, b:b + 1],
                                            in0=c1[:, b:b + 1],
                                            in1=cbA_s[:, b:b + 1],
                                            op=mybir.AluOpType.add)
                    # var = S2/N + c1*(2*S1/N + c1)
                    nc.vector.tensor_scalar(
                        out=var[:, b:b + 1], in0=S1, scalar1=2.0 / N,
                        scalar2=None, op0=mybir.AluOpType.mult)
                    nc.vector.tensor_tensor(out=var[:, b:b + 1],
                                            in0=var[:, b:b + 1],
                                            in1=c1[:, b:b + 1],
                                            op=mybir.AluOpType.add)
                    nc.vector.tensor_tensor(out=var[:, b:b + 1],
                                            in0=var[:, b:b + 1],
                                            in1=c1[:, b:b + 1],
                                            op=mybir.AluOpType.mult)
                    nc.vector.tensor_scalar(
                        out=var[:, b:b + 1], in0=S2, scalar1=1.0 / N,
                        scalar2=var[:, b:b + 1], op0=mybir.AluOpType.mult,
                        op1=mybir.AluOpType.add)
                    # rstd = 1/sqrt(var + eps)
                    nc.scalar.activation(
                        out=rstd[:, b:b + 1], in_=var[:, b:b + 1],
                        func=mybir.ActivationFunctionType.Sqrt, bias=eps_t[:])
                    nc.vector.reciprocal(rstd[:, b:b + 1], rstd[:, b:b + 1])
                    nc.vector.tensor_tensor(out=f_[:, b:b + 1],
                                            in0=rstd[:, b:b + 1],
                                            in1=gnw_s[:, b:b + 1],
                                            op=mybir.AluOpType.mult)
                    # h = (out + c1) * f + gnb  (into hT)
                    nc.vector.tensor_scalar(
                        out=hT[b][:], in0=outT[b][:],
                        scalar1=c1[:, b:b + 1], scalar2=None,
                        op0=mybir.AluOpType.add)
                    nc.vector.tensor_scalar(
                        out=hT[b][:], in0=hT[b][:],
                        scalar1=f_[:, b:b + 1], scalar2=gnb_s[:, b:b + 1],
                        op0=mybir.AluOpType.mult, op1=mybir.AluOpType.add)
                    # elu(h) - 1 fold: h' = relu(h) + exp(min(h,0))
                    nc.vector.tensor_scalar(
                        out=sq[:], in0=hT[b][:], scalar1=0.0, scalar2=None,
                        op0=mybir.AluOpType.min)
                    nc.scalar.activation(
                        out=sq[:], in_=sq[:],
                        func=mybir.ActivationFunctionType.Exp)
                    nc.vector.tensor_scalar(
                        out=hT[b][:], in0=hT[b][:], scalar1=0.0, scalar2=None,
                        op0=mybir.AluOpType.max)
                    nc.vector.tensor_tensor(
                        out=hT[b][:], in0=hT[b][:], in1=sq[:],
                        op=mybir.AluOpType.add)

                # skip matmul: xT += W_sk.T h' + bsk_eff
                for bo in range(2):
                    for ch0 in range(0, NL, 512):
                        cw = min(512, NL - ch0)
                        ps = psp.tile([128, 512], fp32, tag="wsum", name="wsum")
                        for bi in range(2):
                            nc.tensor.matmul(
                                out=ps[:, :cw],
                                lhsT=mmT(wsk_s[:, bi, bo]),
                                rhs=mmT(hT[bi][:, ch0:ch0 + cw]),
                                start=(bi == 0), stop=(bi == 1))
                        nc.vector.tensor_tensor(
                            out=xT[bo][:, ch0:ch0 + cw],
                            in0=xT[bo][:, ch0:ch0 + cw], in1=ps[:, :cw],
                            op=mybir.AluOpType.add)
                    nc.vector.tensor_scalar(
                        out=xT[bo][:], in0=xT[bo][:],
                        scalar1=bsk_s[:, bo:bo + 1], scalar2=None,
                        op0=mybir.AluOpType.add)

            fcw_s = pp.tile([128, 2], bf16, tag="fcws", name="fcws")
            fcb_s = pp.tile([1, 1], fp32, tag="fcbs", name="fcbs")
            nc.sync.dma_start(out=fcw_s[:], in_=fcw_t.rearrange("b p o -> p (b o)"))
            nc.sync.dma_start(out=fcb_s[:], in_=fcb[:])

            for l in range(n_layers):
                layer(l)

            # final fc
            y_sb = sp.tile([1, NL], fp32, tag="ysb", name="ysb")
            for ch0 in range(0, NL, 512):
                cw = min(512, NL - ch0)
                ps = psp.tile([128, 512], fp32, tag="lg", name="lg", bufs=2)
                for b in range(2):
                    nc.tensor.matmul(out=ps[:1, :cw], lhsT=mmT(fcw_s[:, b:b + 1]),
                                     rhs=mmT(xT[b][:, ch0:ch0 + cw]),
                                     start=(b == 0), stop=(b == 1))
                nc.scalar.activation(
                    out=y_sb[:, ch0:ch0 + cw], in_=ps[:1, :cw],
                    func=mybir.ActivationFunctionType.Identity,
                    bias=fcb_s[:], scale=1.0)
            nc.sync.dma_start(out=y_out[:], in_=y_sb[:])

    nc.compile()
    return nc


# ---------------------------------------------------------------- runner

_CACHE = {}


def kernel(**inputs):
    layout, in_maps = prep_inputs(inputs)
    key = (layout["NL"], layout["E_pad"],
           tuple(int(x) for x in layout["tile_edge_base"]))
    if key not in _CACHE:
        _CACHE[key] = build_program(layout)
    nc = _CACHE[key]
    res = bass_utils.run_bass_kernel_spmd(nc, in_maps, core_ids=list(range(NCORES)))
    y = np.zeros(N, np.float32)
    for c in range(NCORES):
        yc = np.asarray(res.results[c]["y"], np.float32).reshape(-1)
        y[layout["core_nodes"][c]] = yc[layout["cores"][c]["slot_of_node"]]
    return y


if __name__ == "__main__":
    sys.path.insert(0, "/root/problem")
    import jax
    import reference

    with jax.default_device(jax.devices("cpu")[0]):
        inputs = {k: np.asarray(v) for k, v in reference.setup_inputs().items()}
        expected = np.asarray(reference.reference(**inputs))
    got = kernel(**inputs)
    rel = np.linalg.norm(got - expected) / np.linalg.norm(expected)
    print("rel l2:", rel)
    print(expected[:4], got[:4])

